# revision 25
# baseline (speedup 1.0000x reference)
"""AlternatingFlow (16 planar/radial flow layers) on 8 Trainium2 NeuronCores.

Strategy: data-parallel over the batch (131072 rows -> 8 x 16384).
Each core runs the full 16-layer flow on its shard, keeping z resident in
SBUF in [128, blk, 256] layout. Per-row scalars needed for the loss (tanh
activations for planar layers, squared radii for radial layers) are tiny
side outputs; the loss (which needs the global full-tensor radius norm R)
is finalized on the host from those per-row scalars.
"""

import numpy as np

import concourse.bass as bass
import concourse.bacc as bacc
import concourse.mybir as mybir
from concourse.tile import TileContext
from concourse.bass_utils import run_bass_kernel_spmd

L = 16
B = 131072
D = 256
EPS = 1e-7
NCORES = 8
BSH = B // NCORES          # 16384 rows per core
NGRP = 4                   # row groups per core
GROWS = BSH // NGRP        # 4096 rows per group
NBLK = GROWS // 128        # 32 blocks of 128 rows
NPLANAR = 8
NRADIAL = 8

F32 = mybir.dt.float32
Alu = mybir.AluOpType
Act = mybir.ActivationFunctionType

_CACHE = {}
TRACE = False
LAST_EXEC_NS = None
LAST_WALL_NS = None


def _build_nc():
    nc = bacc.Bacc("TRN2", target_bir_lowering=False, debug=False)

    z_in = nc.dram_tensor("z", [BSH, D], F32, kind="ExternalInput")
    wb = nc.dram_tensor("wb", [128, NPLANAR, D], F32, kind="ExternalInput")
    sb = nc.dram_tensor("sb", [128, NPLANAR, D], F32, kind="ExternalInput")
    z0b = nc.dram_tensor("z0b", [128, NRADIAL, D], F32, kind="ExternalInput")
    pb = nc.dram_tensor("pb", [128, NPLANAR], F32, kind="ExternalInput")
    ar = nc.dram_tensor("ar", [128, NRADIAL], F32, kind="ExternalInput")
    br = nc.dram_tensor("br", [128, NRADIAL], F32, kind="ExternalInput")

    z_out = nc.dram_tensor("zout", [BSH, D], F32, kind="ExternalOutput")
    acts_out = nc.dram_tensor("acts", [NPLANAR, NGRP, 128, NBLK], F32,
                              kind="ExternalOutput")
    r2s_out = nc.dram_tensor("r2s", [NRADIAL, NGRP, 128, NBLK], F32,
                             kind="ExternalOutput")

    with TileContext(nc) as tc:
        with (
            tc.tile_pool(name="params", bufs=1) as ppool,
            tc.tile_pool(name="zpool", bufs=2) as zpool,
            tc.tile_pool(name="scratch", bufs=1) as spool,
            tc.tile_pool(name="small", bufs=2) as mpool,
        ):
            # Load per-layer parameters once (kept resident).
            wb_t = ppool.tile([128, NPLANAR, D], F32, tag="wb")
            nc.sync.dma_start(wb_t[:, :, :], wb[:, :, :])
            sb_t = ppool.tile([128, NPLANAR, D], F32, tag="sb")
            nc.sync.dma_start(sb_t[:, :, :], sb[:, :, :])
            z0_t = ppool.tile([128, NRADIAL, D], F32, tag="z0")
            nc.sync.dma_start(z0_t[:, :, :], z0b[:, :, :])
            pb_t = ppool.tile([128, NPLANAR], F32, tag="pb")
            nc.sync.dma_start(pb_t[:, :], pb[:, :])
            ar_t = ppool.tile([128, NRADIAL], F32, tag="ar")
            nc.sync.dma_start(ar_t[:, :], ar[:, :])
            br_t = ppool.tile([128, NRADIAL], F32, tag="br")
            nc.sync.dma_start(br_t[:, :], br[:, :])

            # Touch each param tile once on its consuming engine so later
            # consumers inherit the DMA dependency via program order (a
            # TensorTensor instruction only supports one hw sync-wait).
            for t_ in (wb_t, sb_t, z0_t):
                nc.vector.tensor_scalar_mul(t_[:, :, :], t_[:, :, :], 1.0)
            for t_ in (ar_t, br_t):
                nc.vector.tensor_scalar_mul(t_[:, :], t_[:, :], 1.0)
            nc.scalar.mul(pb_t[:, :], pb_t[:, :], 1.0)

            for g in range(NGRP):
                rows = slice(g * GROWS, (g + 1) * GROWS)
                zt = zpool.tile([128, NBLK, D], F32, tag="zt")
                nc.sync.dma_start(
                    zt[:, :, :],
                    z_in[rows, :].rearrange("(p blk) d -> p blk d", p=128),
                )
                # Funnel the DMA dependency through a tensor_scalar touch:
                # TensorTensor instructions only support one hw sync-wait.
                nc.vector.tensor_scalar_mul(zt[:, :, :], zt[:, :, :], 1.0)

                prod = None
                for i in range(L):
                    k = i // 2
                    prod = spool.tile([128, NBLK, D], F32, tag="prod")
                    if i % 2 == 0:
                        # planar: act = tanh(z @ w + b); z += s * act
                        w_ap = wb_t[:, k:k + 1, :].broadcast_to([128, NBLK, D])
                        s_ap = sb_t[:, k:k + 1, :].broadcast_to([128, NBLK, D])
                        nc.vector.tensor_tensor(
                            prod[:, :, :], zt[:, :, :], w_ap, op=Alu.mult)
                        pre = mpool.tile([128, NBLK], F32, tag="pre")
                        nc.vector.tensor_reduce(
                            pre[:, :], prod[:, :, :],
                            axis=mybir.AxisListType.X, op=Alu.add)
                        act = mpool.tile([128, NBLK], F32, tag="act")
                        nc.scalar.activation(
                            act[:, :], pre[:, :], Act.Tanh,
                            bias=pb_t[:, k:k + 1], scale=1.0)
                        nc.sync.dma_start(acts_out[k, g, :, :], act[:, :])
                        act_ap = act[:, :].rearrange(
                            "p (blk o) -> p blk o", o=1).broadcast_to([128, NBLK, D])
                        nc.vector.tensor_tensor(
                            prod[:, :, :], s_ap, act_ap, op=Alu.mult)
                        nc.vector.tensor_tensor(
                            zt[:, :, :], zt[:, :, :], prod[:, :, :],
                            op=Alu.add)
                    else:
                        # radial: rad = z - z0; r = |rad|; h = 1/(a+r);
                        # z += (beta*h) * rad
                        z0_ap = z0_t[:, k:k + 1, :].broadcast_to([128, NBLK, D])
                        nc.vector.tensor_tensor(
                            prod[:, :, :], zt[:, :, :], z0_ap,
                            op=Alu.subtract)
                        sq = spool.tile([128, NBLK, D], F32, tag="sq")
                        nc.scalar.activation(
                            sq[:, :, :], prod[:, :, :], Act.Square)
                        r2 = mpool.tile([128, NBLK], F32, tag="r2")
                        nc.vector.tensor_reduce(
                            r2[:, :], sq[:, :, :],
                            axis=mybir.AxisListType.X, op=Alu.add)
                        nc.sync.dma_start(r2s_out[k, g, :, :], r2[:, :])
                        r0 = mpool.tile([128, NBLK], F32, tag="r0")
                        nc.scalar.activation(r0[:, :], r2[:, :], Act.Sqrt)
                        # ACT sqrt is low-precision; one Newton step:
                        # r = 0.5*(r0 + r2/r0)
                        ir0 = mpool.tile([128, NBLK], F32, tag="ir0")
                        nc.vector.reciprocal(ir0[:, :], r0[:, :])
                        rq = mpool.tile([128, NBLK], F32, tag="rq")
                        nc.vector.tensor_tensor(
                            rq[:, :], r2[:, :], ir0[:, :], op=Alu.mult)
                        r = mpool.tile([128, NBLK], F32, tag="r")
                        nc.vector.tensor_tensor(
                            r[:, :], r0[:, :], rq[:, :], op=Alu.add)
                        t = mpool.tile([128, NBLK], F32, tag="t")
                        nc.vector.tensor_scalar(
                            t[:, :], r[:, :], 0.5, ar_t[:, k:k + 1],
                            op0=Alu.mult, op1=Alu.add)
                        h = mpool.tile([128, NBLK], F32, tag="h")
                        nc.vector.reciprocal(h[:, :], t[:, :])
                        c = mpool.tile([128, NBLK], F32, tag="c")
                        nc.vector.tensor_scalar(
                            c[:, :], h[:, :], br_t[:, k:k + 1], None,
                            op0=Alu.mult)
                        c_ap = c[:, :].rearrange(
                            "p (blk o) -> p blk o", o=1).broadcast_to([128, NBLK, D])
                        nc.vector.tensor_tensor(
                            sq[:, :, :], prod[:, :, :], c_ap, op=Alu.mult)
                        nc.vector.tensor_tensor(
                            zt[:, :, :], zt[:, :, :], sq[:, :, :],
                            op=Alu.add)

                nc.sync.dma_start(
                    z_out[rows, :].rearrange("(p blk) d -> p blk d", p=128),
                    zt[:, :, :],
                )
    nc.compile()
    return nc


def _host_params(planar_w, planar_b, planar_s, radial_z0, radial_a, radial_b):
    """Fold the reference's parameter corrections (pure fp32, faithful)."""
    w = np.empty((NPLANAR, D), np.float32)
    s = np.empty((NPLANAR, D), np.float32)
    pbias = np.empty(NPLANAR, np.float32)
    z0 = np.empty((NRADIAL, D), np.float32)
    av = np.empty(NRADIAL, np.float32)
    bv = np.empty(NRADIAL, np.float32)
    for k in range(NPLANAR):
        i = 2 * k
        wi = planar_w[i].astype(np.float32)
        si = planar_s[i].astype(np.float32)
        margin = np.float32(np.dot(si, wi))
        comp = np.float32(-1.0) + np.log1p(np.exp(margin)) - margin
        if margin < -1.0:
            si = si + comp * wi / np.float32(np.linalg.norm(wi))
        w[k] = wi
        s[k] = si
        pbias[k] = planar_b[i]
    for k in range(NRADIAL):
        i = 2 * k + 1
        z0[k] = radial_z0[i]
        a_i = np.float32(radial_a[i])
        b_i = np.float32(radial_b[i])
        if b_i < -a_i:
            b_i = -a_i + np.log1p(np.exp(b_i))
        av[k] = a_i
        bv[k] = b_i
    return w, s, pbias, z0, av, bv


def kernel(z, planar_w, planar_b, planar_s, radial_z0, radial_a, radial_b):
    z = np.ascontiguousarray(z, dtype=np.float32)
    w, s, pbias, z0, av, bv = _host_params(
        planar_w, planar_b, planar_s, radial_z0, radial_a, radial_b)

    wb = np.broadcast_to(w[None, :, :], (128, NPLANAR, D)).copy()
    sb = np.broadcast_to(s[None, :, :], (128, NPLANAR, D)).copy()
    z0b = np.broadcast_to(z0[None, :, :], (128, NRADIAL, D)).copy()
    pb = np.broadcast_to(pbias[None, :], (128, NPLANAR)).copy()
    ar = np.broadcast_to(av[None, :], (128, NRADIAL)).copy()
    br = np.broadcast_to(bv[None, :], (128, NRADIAL)).copy()

    if "nc" not in _CACHE:
        _CACHE["nc"] = _build_nc()
    nc = _CACHE["nc"]

    in_maps = []
    for c in range(NCORES):
        in_maps.append(dict(
            z=z[c * BSH:(c + 1) * BSH],
            wb=wb, sb=sb, z0b=z0b, pb=pb, ar=ar, br=br,
        ))
    import time as _time
    _t0 = _time.time()
    try:
        res = run_bass_kernel_spmd(nc, in_maps, core_ids=list(range(NCORES)),
                                   trace=TRACE)
    except ModuleNotFoundError:
        # NTFF profile hook unavailable under this axon build.
        res = run_bass_kernel_spmd(nc, in_maps, core_ids=list(range(NCORES)))
    _t1 = _time.time()
    global LAST_EXEC_NS, LAST_WALL_NS
    LAST_EXEC_NS = res.exec_time_ns
    LAST_WALL_NS = int((_t1 - _t0) * 1e9)
    outs = res.results

    z_full = np.concatenate([o["zout"] for o in outs], axis=0)

    # acts/r2s tiles [k, g, p, blk] -> row index g*GROWS + p*NBLK + blk,
    # which is exactly flat [k, g*128*NBLK + p*NBLK + blk] order.
    def detile(arrs):
        # arrs: per-core [K, NGRP, 128, NBLK] -> [K, B]
        per_core = [a.reshape(a.shape[0], BSH) for a in arrs]
        return np.concatenate(per_core, axis=1)

    acts = detile([o["acts"] for o in outs])
    r2s = detile([o["r2s"] for o in outs])

    # Host loss finalization (per-row scalars only), mirroring the
    # reference's fp32 arithmetic step by step.
    one = np.float32(1.0)
    eps = np.float32(EPS)
    loss = np.zeros(B, np.float32)
    for k in range(NPLANAR):
        act = acts[k]
        wds = np.float32(np.dot(w[k], s[k]))
        det = one + (one - act * act) * wds
        loss = loss + np.log(np.abs(det) + eps, dtype=np.float32)
    for k in range(NRADIAL):
        r2 = r2s[k]
        R = np.float32(np.sqrt(r2.astype(np.float64).sum()))
        r = np.sqrt(r2)
        h = one / (av[k] + r)
        bh = bv[k] * h
        diagonal = np.power(one + bh, np.float32(D - 1))
        det = diagonal * (one + bh - bv[k] * (h * h) * R)
        loss = loss + np.log(np.abs(det) + eps, dtype=np.float32)
    loss_mean = np.float32(loss.astype(np.float64).mean())

    return z_full, loss_mean
